# revision 7
# baseline (speedup 1.0000x reference)
"""CLRHead forward, 8-way batch-data-parallel on trn2 NeuronCores.

Sharding: batch B=64 -> 8 cores x 8; all params replicated; no cross-core
communication (pure data parallelism per the problem's structure).
"""
import sys
import os

sys.path.insert(0, "/opt/trn_rl_repo")

import numpy as np
import jax
import jax.numpy as jnp
from functools import partial

# ---- hardcoded problem constants (input-independent) ----
P, S, NOFF, NSTRIP = 192, 36, 72, 71
C, HID = 64, 64
IMG_W, IMG_H = 640.0, 512.0
B_TOTAL = 64
N_CORES = 8
B_LOCAL = B_TOTAL // N_CORES

SAMPLE_X = (np.linspace(0.0, 1.0, S, dtype=np.float32) * NSTRIP).astype(np.int32)
PRIOR_FEAT_YS = np.ascontiguousarray((1.0 - SAMPLE_X.astype(np.float32) / NSTRIP)[::-1])
PRIOR_YS = np.linspace(1.0, 0.0, NOFF, dtype=np.float32)

# nearest-resize gather indices for each stage's feature map -> (10, 25)
_RESIZE = {}
for _H, _W in ((64, 80), (32, 40), (16, 20)):
    iy = (np.arange(10) * _H // 10).astype(np.int32)
    ix = (np.arange(25) * _W // 25).astype(np.int32)
    _RESIZE[(_H, _W)] = (iy, ix)


# --- gather-free helpers (neuronx-cc chokes on indirect loads; use dense matmuls) ---

def _tent_rows(ys, H):
    # constant bilinear row-weight matrix (S, H): tri(y_s - h)
    d = np.abs(ys[:, None] * (H - 1) - np.arange(H, dtype=np.float32)[None, :])
    return np.maximum(0.0, 1.0 - d).astype(np.float32)

_RY = {64: _tent_rows(PRIOR_FEAT_YS, 64),
       32: _tent_rows(PRIOR_FEAT_YS, 32),
       16: _tent_rows(PRIOR_FEAT_YS, 16)}

# one-hot selector for priors_on_fm with the sample flip folded in: (78, S)
_SEL = np.zeros((6 + NOFF, S), np.float32)
for _j, _sx in enumerate(SAMPLE_X[::-1]):
    _SEL[6 + _sx, _j] = 1.0

# one-hot resize-nearest selectors
_GY = {}
_GX = {}
for _H, _W in ((64, 80), (32, 40), (16, 20)):
    gy_ = np.zeros((_H, 10), np.float32)
    gx_ = np.zeros((_W, 25), np.float32)
    for _o, _i in enumerate((np.arange(10) * _H // 10)):
        gy_[_i, _o] = 1.0
    for _o, _i in enumerate((np.arange(25) * _W // 25)):
        gx_[_i, _o] = 1.0
    _GY[_H] = gy_
    _GX[_W] = gx_


def _grid_sample_dense(fmap, xnorm):
    # fmap (b,C,H,W); xnorm (b,P,S) normalized x in [0,1] (prior_xs values).
    # y coords are the fixed PRIOR_FEAT_YS per s. Bilinear w/ zeros padding +
    # align_corners=True == tent weights relu(1-|x_pix - w|) for ALL x.
    b, Cc, H, W = fmap.shape
    x_pix = xnorm * (W - 1)
    tx = jax.nn.relu(1.0 - jnp.abs(
        x_pix[..., None] - jnp.arange(W, dtype=jnp.float32)))      # (b,P,S,W)
    t1 = jnp.einsum('bchw,sh->bcsw', fmap, jnp.asarray(_RY[H]))     # (b,C,S,W)
    return jnp.einsum('bcsw,bpsw->bcps', t1, tx)                    # (b,C,P,S)


def _conv1d(x, w, pad):
    return jax.lax.conv_general_dilated(x, w, window_strides=(1,), padding=[(pad, pad)],
                                        dimension_numbers=('NCH', 'OIH', 'NCH'))


def _layernorm(x, g, bta):
    mu = jnp.mean(x, axis=-1, keepdims=True)
    var = jnp.mean((x - mu) ** 2, axis=-1, keepdims=True)
    return (x - mu) / jnp.sqrt(var + 1e-5) * g + bta


def _forward_local(feat0, feat1, feat2, priors, convs_w, convs_scale, convs_shift,
                   cat_w0, cat_w1, cat_w2, cat_scale, cat_shift,
                   fkey_w, fkey_scale, fkey_shift, fval_w, fval_b,
                   fq_w, fq_b, attW_w, attW_b, fc_w, fc_b, ln_g, ln_b,
                   cls_mlp_w, cls_mlp_b, reg_mlp_w, reg_mlp_b,
                   cls_head_w, cls_head_b, reg_head_w, reg_head_b):
    feats = [feat0, feat1, feat2]
    cat_ws = [cat_w0, cat_w1, cat_w2]
    b = feat0.shape[0]
    prior_ys = jnp.asarray(PRIOR_YS)
    feat_ys = jnp.asarray(PRIOR_FEAT_YS)
    priors_b = jnp.broadcast_to(priors[None], (b, P, 6 + NOFF))
    sel = jnp.asarray(_SEL)
    prior_xs = jnp.einsum('bpf,fs->bps', priors_b, sel)   # gather+flip as matmul
    cfs = []          # cached per-stage conv outputs (reference recomputes; identical values)
    preds_list = []
    for stage in range(3):
        fmap = feats[stage]
        pooled = _grid_sample_dense(fmap, prior_xs)                 # (b,C,P,S)
        roi = pooled.transpose(0, 2, 1, 3).reshape(b * P, C, S)
        cfs.append(jax.nn.relu(_conv1d(roi, convs_w[stage], 4)
                               * convs_scale[stage][None, :, None]
                               + convs_shift[stage][None, :, None]))
        cat = jnp.concatenate(cfs[:stage + 1], axis=1)
        cat = jax.nn.relu(_conv1d(cat, cat_ws[stage], 4)
                          * cat_scale[stage][None, :, None] + cat_shift[stage][None, :, None])
        roi_flat = cat.reshape(b * P, C * S)
        roi_fc = jax.nn.relu(_layernorm(roi_flat @ fc_w.T + fc_b, ln_g, ln_b)).reshape(b, P, HID)
        # attention: nearest-resize commutes with the 1x1 convs (exact same floats),
        # so select the 250 pixels first (as one-hot matmuls) and run the
        # pointwise convs on those only.
        H, W = fmap.shape[2], fmap.shape[3]
        small = jnp.einsum('bchw,hy,wx->bcyx', fmap,
                           jnp.asarray(_GY[H]), jnp.asarray(_GX[W])).reshape(b, C, 250)
        value = jnp.einsum('bck,oc->bok', small, fval_w) + fval_b[None, :, None]
        keyf = jax.nn.relu(jnp.einsum('bck,oc->bok', small, fkey_w)
                           * fkey_scale[None, :, None] + fkey_shift[None, :, None])
        query = jax.nn.relu(roi_fc * fq_w[None, :, None] + fq_b[None, :, None])
        sim = jax.nn.softmax(jnp.einsum('bpc,bck->bpk', query, keyf) * (C ** -0.5), axis=-1)
        ctx = jnp.einsum('bpk,bck->bpc', sim, value)
        ctx = ctx * attW_w[None, :, None] + attW_b[None, :, None]
        fc_feat = (roi_fc + ctx).reshape(b * P, HID)
        clsf, regf = fc_feat, fc_feat
        for j in range(2):
            clsf = jax.nn.relu(clsf @ cls_mlp_w[j].T + cls_mlp_b[j])
            regf = jax.nn.relu(regf @ reg_mlp_w[j].T + reg_mlp_b[j])
        cls_logits = (clsf @ cls_head_w.T + cls_head_b).reshape(b, P, 2)
        # split the reg head into separate matmuls: avoids slicing a traced
        # (b,P,76) tensor, which tickles a neuronx-cc tensorizer bug
        r3 = (regf @ reg_head_w[:3].T + reg_head_b[:3]).reshape(b, P, 3)
        p5 = (regf @ reg_head_w[3:4].T + reg_head_b[3:4]).reshape(b, P, 1)
        r_off = (regf @ reg_head_w[4:].T + reg_head_b[4:]).reshape(b, P, NOFF)
        p25 = priors_b[:, :, 2:5] + r3
        pa = p25[:, :, 0]
        pb = p25[:, :, 1]
        pth = p25[:, :, 2]
        inv_tan = 1.0 / jnp.tan(pth * np.pi + 1e-5)
        offs = (pb[:, :, None] * (IMG_W - 1)
                + (1.0 - prior_ys[None, None, :] - pa[:, :, None]) * IMG_H
                * inv_tan[:, :, None]) / (IMG_W - 1)
        preds = jnp.concatenate([cls_logits, p25, p5, offs + r_off], axis=-1)
        preds_list.append(preds)
        if stage != 2:
            lines = jnp.concatenate([cls_logits, p25, p5, offs], axis=-1)
            priors_b = lines
            prior_xs = jnp.einsum('bpf,fs->bps', priors_b, sel)
    return jnp.stack(preds_list)  # (3, b, P, 78)


_PMAPPED = None


def _get_pmapped():
    global _PMAPPED
    if _PMAPPED is None:
        # batch args sharded on axis 0; everything else replicated
        in_axes = (0, 0, 0) + (None,) * 30
        _PMAPPED = jax.pmap(_forward_local, in_axes=in_axes,
                            devices=jax.devices()[:N_CORES])
    return _PMAPPED


def kernel(**inputs):
    f = _get_pmapped()
    feat0 = np.ascontiguousarray(inputs['feat0'], dtype=np.float32).reshape(
        N_CORES, B_LOCAL, C, 64, 80)
    feat1 = np.ascontiguousarray(inputs['feat1'], dtype=np.float32).reshape(
        N_CORES, B_LOCAL, C, 32, 40)
    feat2 = np.ascontiguousarray(inputs['feat2'], dtype=np.float32).reshape(
        N_CORES, B_LOCAL, C, 16, 20)
    order = ['priors', 'convs_w', 'convs_scale', 'convs_shift',
             'cat_w0', 'cat_w1', 'cat_w2', 'cat_scale', 'cat_shift',
             'fkey_w', 'fkey_scale', 'fkey_shift', 'fval_w', 'fval_b',
             'fq_w', 'fq_b', 'attW_w', 'attW_b', 'fc_w', 'fc_b', 'ln_g', 'ln_b',
             'cls_mlp_w', 'cls_mlp_b', 'reg_mlp_w', 'reg_mlp_b',
             'cls_head_w', 'cls_head_b', 'reg_head_w', 'reg_head_b']
    rest = [np.asarray(inputs[k], dtype=np.float32) for k in order]
    out = f(feat0, feat1, feat2, *rest)      # (8, 3, 8, 192, 78)
    out = np.asarray(out)
    return out.transpose(1, 0, 2, 3, 4).reshape(3, B_TOTAL, P, 6 + NOFF)
